# revision 10
# baseline (speedup 1.0000x reference)
"""Inverse separable wavelet synthesis (stride-2 transposed conv, 9 taps,
36 -> 12 -> 4 channels, 256x256 -> 512x512) on 8 trn2 NeuronCores.

Formulation: both passes are dense matmuls against host-precomputed banded
operator matrices A_beta [256 in, 512 out] (one per wavelet band), with
symmetric padding + border-mask sign folded into the operators.  All filter
coefficients are dyadic rationals with <= 8 mantissa bits -> EXACT in bf16,
so everything on-chip runs in bf16 (input and intermediates round to bf16;
PSUM accumulation stays exact fp32).

Host side: input is permuted to [b, h, by, g2, bx, w] (band 'by' outermost
of the channel split c = 9*g2 + 3*by + bx) and cast to bf16.  This makes
every matmul moving-operand access pattern contiguous in 256-byte runs
(full 16B-SBUF-cacheline hits) and halves input DMA bytes.

Input DMA descriptors are split to 4608 B (max_dma_last_dim) so each load
instruction carries 272 descriptors -> the HWDGE spreads them over all 16
SDMA engines (68 x 18KB descriptors land on only 4 engines = the previous
bottleneck).

Per-core pipeline (pure batch parallelism, 2 images per core), fully
streamed per 128-row h2-block:
  load x window [68 h, (by g2 bx w)]  (one DMA, 272 descriptors)
  stage Y : lhsT = A_by window [68, 128 h2], moving = x win [68, (g2, w)]
            -> psY [h2, (g2, w-half)] (3-band accumulation), cast -> u bf16
  PE transpose u [h2, (q, w)] -> up [w, (q, h2)]      (q = 3*g2 + bx)
  stage X : lhsT = A_bx block [128 w, 128 w2], moving = up [w, (g2, h2)]
            -> psX [w2, (g2, h2)], cast -> v bf16
  PE transpose v [w2, (c, h2)] -> osb [h2, (w2, c)] fp32 -> DRAM rows
"""

import numpy as np
import ml_dtypes
from contextlib import ExitStack

import concourse.bass as bass
import concourse.bacc as bacc
import concourse.mybir as mybir
import concourse.tile as tile
from concourse.bass_utils import run_bass_kernel_spmd

B, H, W, C = 16, 256, 256, 36
NCORES = 8
BPC = B // NCORES  # batches per core
W2 = 2 * W
H2 = 2 * H
F32 = mybir.dt.float32
BF16 = mybir.dt.bfloat16

SMOOTH = [0.0, 0.0, 1.0 / 16.0, 0.5, 14.0 / 16.0, 0.5, 1.0 / 16.0, 0.0, 0.0]
EVEN = [-1.0 / 128.0, -1.0 / 16.0, -10.0 / 64.0, -7.0 / 16.0, 85.0 / 64.0,
        -7.0 / 16.0, -10.0 / 64.0, -1.0 / 16.0, -1.0 / 128.0]
ODD = [1.0 / 256.0, 1.0 / 32.0, 15.0 / 128.0, 17.0 / 32.0, 0.0,
       -17.0 / 32.0, -15.0 / 128.0, -1.0 / 32.0, -1.0 / 256.0]

# Stage X: which 128-row k-tiles of up feed each 128-col w2 output block
# (out block n covers in rows [64n-2, 64n+65]).
KTS = {0: (0,), 1: (0, 1), 2: (0, 1), 3: (1,)}
# Stage Y input windows (one 70-row window per 128-row h2 output block).
# Uniform stride 62 lets ONE DMA instruction load all 4 windows (280
# descriptors -> spread over all 16 SDMA engines; 68-descriptor loads
# land on only 4 engines and bottleneck the kernel).
W0 = [0, 62, 124, 186]
KW = 70


def _build_operator_full():
    """[3 bands, 256 in-rows, 512 out-cols] float64 folded operator."""
    inv = np.array([SMOOTH, EVEN, ODD], dtype=np.float64)
    S = 256
    Sp = S + 6
    j = np.arange(Sp)[:, None]
    m = np.arange(2 * S)[None, :]
    t = m + 10 - 2 * j
    valid = (t >= 0) & (t <= 8)
    P = np.zeros((3, Sp, 2 * S))
    for b in range(3):
        P[b][valid] = inv[b][t[valid]]
    # border mask: odd band negated on the 3-wide padded border
    P[2, [0, 1, 2, Sp - 3, Sp - 2, Sp - 1], :] *= -1.0
    # fold symmetric padding: pad[0..2]=x[2],x[1],x[0]; pad[-3:]=x[-1],x[-2],x[-3]
    A = P[:, 3:3 + S].copy()
    A[:, 2] += P[:, 0]
    A[:, 1] += P[:, 1]
    A[:, 0] += P[:, 2]
    A[:, S - 1] += P[:, Sp - 3]
    A[:, S - 2] += P[:, Sp - 2]
    A[:, S - 3] += P[:, Sp - 1]
    return A


def _build_operator_array():
    """Stage-X operator: [3 bands, 2 ktiles, 128 in-rows, 512 out-cols] bf16."""
    A = _build_operator_full()
    return np.ascontiguousarray(
        A.reshape(3, 2, 128, 512).astype(ml_dtypes.bfloat16))


def _build_operator_windows():
    """Stage-Y operator: [3 bands, 4 blocks, 68 in-rows, 128 out-cols] bf16."""
    A = _build_operator_full()
    out = np.zeros((3, 4, KW, 128), np.float64)
    for blk in range(4):
        out[:, blk] = A[:, W0[blk]:W0[blk] + KW, blk * 128:(blk + 1) * 128]
    return np.ascontiguousarray(out.astype(ml_dtypes.bfloat16))


def _build_program(repeat=1):
    nc = bacc.Bacc("TRN2", target_bir_lowering=False)
    # x: [b, p, win, row(+pad)] bf16 — host-materialized overlapping stage-Y
    # windows (row = (by, g2, bx, w) channel-permuted).  Row-interleaved
    # window layout keeps the DMA descriptor stream sequential in DRAM;
    # the 128 B pad stops descriptor coalescing so one load = 280 x 18 KB
    # descriptors -> all 16 SDMA engines at full rate.
    ROW = 3 * 12 * W
    PADW = 64
    x = nc.declare_dram_parameter("x", [BPC, KW, 4, ROW + PADW], BF16,
                                  isOutput=False)
    a_w = nc.declare_dram_parameter("a_w", [3, 4, KW, 128], BF16,
                                    isOutput=False)
    a_op = nc.declare_dram_parameter("a_op", [3, 2, 128, W2], BF16,
                                     isOutput=False)
    ident = nc.declare_dram_parameter("ident", [128, 128], BF16,
                                      isOutput=False)
    out = nc.declare_dram_parameter("out", [BPC, H2, W2, 4], F32,
                                    isOutput=True)

    with tile.TileContext(nc) as tc, ExitStack() as ctx:
        const = ctx.enter_context(tc.tile_pool(name="const", bufs=1))
        xpool = ctx.enter_context(tc.tile_pool(name="xp", bufs=1))
        upool = ctx.enter_context(tc.tile_pool(name="up", bufs=2))
        wpool = ctx.enter_context(tc.tile_pool(name="wp", bufs=2))
        vpool = ctx.enter_context(tc.tile_pool(name="vp", bufs=2))
        opool = ctx.enter_context(tc.tile_pool(name="op", bufs=3))
        psY = ctx.enter_context(tc.tile_pool(name="psY", bufs=2, space="PSUM"))
        psT = ctx.enter_context(tc.tile_pool(name="psT", bufs=2, space="PSUM"))
        psX = ctx.enter_context(tc.tile_pool(name="psX", bufs=2, space="PSUM"))
        psO = ctx.enter_context(tc.tile_pool(name="psO", bufs=2, space="PSUM"))

        aw_sb = {}
        for beta in range(3):
            for blk in range(4):
                t = const.tile([KW, 128], BF16, name=f"aw_{beta}_{blk}",
                               tag=f"aw_{beta}_{blk}")
                nc.sync.dma_start(t[:], a_w[beta, blk])
                aw_sb[beta, blk] = t
        a_sb = {}
        for beta in range(3):
            for kt in range(2):
                t = const.tile([128, W2], BF16, name=f"a_{beta}_{kt}",
                               tag=f"a_{beta}_{kt}")
                nc.sync.dma_start(t[:], a_op[beta, kt])
                a_sb[beta, kt] = t
        ident_sb = const.tile([128, 128], BF16, name="ident_sb", tag="ident")
        nc.sync.dma_start(ident_sb[:], ident[:])

        for rep in range(repeat):
          for b in range(BPC):
            rb = rep * BPC + b
            # ---- one DMA for all 4 windows: [70, 4, ROW] = 280 descriptors
            xt = xpool.tile([KW, 4 * ROW], BF16, name=f"x_{rb}", tag="x")
            src = x[b, :, :, 0:ROW]
            dst = xt.rearrange("h (win r) -> h win r", win=4)
            eng = nc.sync if b % 2 == 0 else nc.scalar
            eng.dma_start(dst, src)
            for blk in range(4):
                xv = xt.rearrange("h (win by g2 bx w) -> h win by g2 bx w",
                                  win=4, by=3, g2=4, bx=3)[:, blk]

                # ---- stage Y: u[blk] [h2 128, (g2, bx, w)] bf16
                u = upool.tile([128, 12 * W], BF16, name=f"u_{rb}_{blk}",
                               tag="u")
                uv = u.rearrange("p (g2 bx w) -> p g2 bx w", g2=4, bx=3)
                for bx in range(3):
                    for wc in range(2):
                        ps = psY.tile([128, 512], F32,
                                      name=f"psY_{rb}_{blk}_{bx}_{wc}",
                                      tag="psY")
                        psv = ps.rearrange("p (g w) -> p g w", g=4)
                        for i, by in enumerate(range(3)):
                            rhs = xv[:, by, :, bx, wc * 128:(wc + 1) * 128]
                            nc.tensor.matmul(psv, aw_sb[by, blk][:], rhs,
                                             start=(i == 0), stop=(i == 2))
                        dst = uv[:, :, bx, wc * 128:(wc + 1) * 128]
                        if (bx + wc) % 2 == 0:
                            nc.vector.tensor_copy(out=dst, in_=psv)
                        else:
                            nc.scalar.copy(out=dst, in_=psv)

                # ---- mid transpose: u [h2, (q, w)] -> up[wt] [w, (q, h2)]
                up = {}
                for wt in range(2):
                    up[wt] = wpool.tile([128, 12 * 128], BF16,
                                        name=f"up_{rb}_{blk}_{wt}", tag="upt")
                uvb = u.rearrange("p (q w) -> p q w", q=12)
                for wt in range(2):
                    for q4 in range(3):
                        pt = psT.tile([128, 512], BF16,
                                      name=f"psT_{rb}_{blk}_{wt}_{q4}",
                                      tag="psT")
                        for i in range(4):
                            q = q4 * 4 + i
                            nc.tensor.transpose(
                                pt[:, i * 128:(i + 1) * 128],
                                uvb[:, q, wt * 128:(wt + 1) * 128],
                                ident_sb[:])
                        dst = up[wt].rearrange("p (q h) -> p q h", q=12)[
                            :, q4 * 4:(q4 + 1) * 4, :]
                        src2 = pt.rearrange("p (q h) -> p q h", q=4)
                        if (wt + q4) % 2 == 0:
                            nc.vector.tensor_copy(out=dst, in_=src2)
                        else:
                            nc.scalar.copy(out=dst, in_=src2)

                # ---- stage X for this h2 block -> v[w2b] [w2, (c, h2slice)]
                v = {}
                for w2b in range(4):
                    ps = psX.tile([128, 512], F32,
                                  name=f"psX_{rb}_{blk}_{w2b}", tag="psX")
                    psv = ps.rearrange("p (g h) -> p g h", g=4)
                    mms = [(bx, kt) for bx in range(3) for kt in KTS[w2b]]
                    for i, (bx, kt) in enumerate(mms):
                        lhsT = a_sb[bx, kt][:, w2b * 128:(w2b + 1) * 128]
                        # up free order q = 3*g2 + bx -> fix bx, stride g2
                        rhs = up[kt].rearrange(
                            "p (g2 e h) -> p e g2 h", g2=4, e=3)[:, bx, :, :]
                        nc.tensor.matmul(psv, lhsT, rhs,
                                         start=(i == 0),
                                         stop=(i == len(mms) - 1))
                    vt = vpool.tile([128, 4 * 128], BF16,
                                    name=f"v_{rb}_{blk}_{w2b}",
                                    tag=f"v_{w2b}")
                    dst = vt.rearrange("p (c h) -> p c h", c=4)
                    if w2b % 2 == 0:
                        nc.scalar.copy(out=dst, in_=psv)
                    else:
                        nc.vector.tensor_copy(out=dst, in_=psv)
                    v[w2b] = vt

                # ---- output transpose: v [w2, (c, h2)] -> osb [h2, (w2, c)]
                osb = opool.tile([128, W2 * 4], F32, name=f"osb_{rb}_{blk}",
                                 tag="osb")
                osbv = osb.rearrange("p (w c) -> p c w", c=4)
                for w2b in range(4):
                    pt = psO.tile([128, 512], BF16,
                                  name=f"psO_{rb}_{blk}_{w2b}", tag="psO")
                    vv = v[w2b].rearrange("p (c h) -> p c h", c=4)
                    for c in range(4):
                        nc.tensor.transpose(
                            pt[:, c * 128:(c + 1) * 128],
                            vv[:, c, :],
                            ident_sb[:])
                    dst = osbv[:, :, w2b * 128:(w2b + 1) * 128]
                    src2 = pt.rearrange("p (c w) -> p c w", c=4)
                    if w2b % 2 == 0:
                        nc.vector.tensor_copy(out=dst, in_=src2)
                    else:
                        nc.scalar.copy(out=dst, in_=src2)
                dstd = out[b, blk * 128:(blk + 1) * 128, :, :].rearrange(
                    "h w c -> h (w c)")
                nc.sync.dma_start(dstd, osb[:])
    nc.compile()
    return nc


_PROGRAMS = {}


def _get_program(repeat=1):
    if repeat not in _PROGRAMS:
        _PROGRAMS[repeat] = _build_program(repeat)
    return _PROGRAMS[repeat]


def _host_inputs(inputs):
    a4 = _build_operator_array()
    aw = _build_operator_windows()
    identity = np.ascontiguousarray(np.eye(128, dtype=ml_dtypes.bfloat16))
    # [B,H,W,C] c = 9*g2 + 3*by + bx -> rows [B, H, (by, g2, bx, W)] bf16
    xp = inputs.reshape(B, H, W, 4, 3, 3).transpose(0, 1, 4, 3, 5, 2)
    xp = xp.astype(ml_dtypes.bfloat16).reshape(B, H, 3 * 12 * W)
    ROW = 3 * 12 * W
    PADW = 64
    # materialize the 4 overlapping stage-Y windows, row-interleaved + pad
    idx = np.arange(KW)[:, None] + np.asarray(W0)[None, :]  # [KW, 4]
    xw = np.zeros((B, KW, 4, ROW + PADW), dtype=ml_dtypes.bfloat16)
    xw[:, :, :, :ROW] = xp[:, idx, :]
    shards = xw.reshape(NCORES, BPC, KW, 4, ROW + PADW)
    return [{"x": np.ascontiguousarray(shards[c]), "a_op": a4, "a_w": aw,
             "ident": identity} for c in range(NCORES)]


def _run(inputs, trace=False, tmpdir=None, repeat=1):
    """Returns (full output [16,512,512,4], BassKernelResults)."""
    inputs = np.ascontiguousarray(np.asarray(inputs, dtype=np.float32))
    assert inputs.shape == (B, H, W, C), inputs.shape
    nc = _get_program(repeat)
    in_maps = _host_inputs(inputs)
    res = run_bass_kernel_spmd(nc, in_maps, core_ids=list(range(NCORES)),
                               trace=trace, tmpdir=tmpdir)
    outs = [np.asarray(res.results[c]["out"]) for c in range(NCORES)]
    full = np.concatenate(outs, axis=0).astype(np.float32)
    return full, res


def kernel(inputs):
    full, _ = _run(inputs)
    return full


# revision 14
# speedup vs baseline: 1.0290x; 1.0290x over previous
"""Inverse separable wavelet synthesis (stride-2 transposed conv, 9 taps,
36 -> 12 -> 4 channels, 256x256 -> 512x512) on 8 trn2 NeuronCores.

Formulation: both passes are dense matmuls against host-precomputed banded
operator matrices A_beta [256 in, 512 out] (one per wavelet band), with
symmetric padding + border-mask sign folded into the operators.  All filter
coefficients are dyadic rationals with <= 8 mantissa bits -> EXACT in bf16,
so everything on-chip runs in bf16 (input and intermediates round to bf16;
PSUM accumulation stays exact fp32).

Host side: input is permuted to [b, h, by, g2, bx, w] (band 'by' outermost
of the channel split c = 9*g2 + 3*by + bx) and cast to bf16.  This makes
every matmul moving-operand access pattern contiguous in 256-byte runs
(full 16B-SBUF-cacheline hits) and halves input DMA bytes.

Input DMA descriptors are split to 4608 B (max_dma_last_dim) so each load
instruction carries 272 descriptors -> the HWDGE spreads them over all 16
SDMA engines (68 x 18KB descriptors land on only 4 engines = the previous
bottleneck).

Per-core pipeline (pure batch parallelism, 2 images per core), fully
streamed per 128-row h2-block:
  load x window [68 h, (by g2 bx w)]  (one DMA, 272 descriptors)
  stage Y : lhsT = A_by window [68, 128 h2], moving = x win [68, (g2, w)]
            -> psY [h2, (g2, w-half)] (3-band accumulation), cast -> u bf16
  PE transpose u [h2, (q, w)] -> up [w, (q, h2)]      (q = 3*g2 + bx)
  stage X : lhsT = A_bx block [128 w, 128 w2], moving = up [w, (g2, h2)]
            -> psX [w2, (g2, h2)], cast -> v bf16
  PE transpose v [w2, (c, h2)] -> osb [h2, (w2, c)] fp32 -> DRAM rows
"""

import numpy as np
import ml_dtypes
from contextlib import ExitStack

import concourse.bass as bass
import concourse.bacc as bacc
import concourse.mybir as mybir
import concourse.tile as tile
from concourse.bass_utils import run_bass_kernel_spmd

B, H, W, C = 16, 256, 256, 36
NCORES = 8
BPC = B // NCORES  # batches per core
W2 = 2 * W
H2 = 2 * H
F32 = mybir.dt.float32
BF16 = mybir.dt.bfloat16

SMOOTH = [0.0, 0.0, 1.0 / 16.0, 0.5, 14.0 / 16.0, 0.5, 1.0 / 16.0, 0.0, 0.0]
EVEN = [-1.0 / 128.0, -1.0 / 16.0, -10.0 / 64.0, -7.0 / 16.0, 85.0 / 64.0,
        -7.0 / 16.0, -10.0 / 64.0, -1.0 / 16.0, -1.0 / 128.0]
ODD = [1.0 / 256.0, 1.0 / 32.0, 15.0 / 128.0, 17.0 / 32.0, 0.0,
       -17.0 / 32.0, -15.0 / 128.0, -1.0 / 32.0, -1.0 / 256.0]

# Stage X: which 128-row k-tiles of up feed each 128-col w2 output block
# (out block n covers in rows [64n-2, 64n+65]).
KTS = {0: (0,), 1: (0, 1), 2: (0, 1), 3: (1,)}
# Stage Y input windows (one 70-row window per 128-row h2 output block).
# Uniform stride 62 lets ONE DMA instruction load all 4 windows (280
# descriptors -> spread over all 16 SDMA engines; 68-descriptor loads
# land on only 4 engines and bottleneck the kernel).
W0 = [0, 62, 124, 186]
KW = 70


def _build_operator_full():
    """[3 bands, 256 in-rows, 512 out-cols] float64 folded operator."""
    inv = np.array([SMOOTH, EVEN, ODD], dtype=np.float64)
    S = 256
    Sp = S + 6
    j = np.arange(Sp)[:, None]
    m = np.arange(2 * S)[None, :]
    t = m + 10 - 2 * j
    valid = (t >= 0) & (t <= 8)
    P = np.zeros((3, Sp, 2 * S))
    for b in range(3):
        P[b][valid] = inv[b][t[valid]]
    # border mask: odd band negated on the 3-wide padded border
    P[2, [0, 1, 2, Sp - 3, Sp - 2, Sp - 1], :] *= -1.0
    # fold symmetric padding: pad[0..2]=x[2],x[1],x[0]; pad[-3:]=x[-1],x[-2],x[-3]
    A = P[:, 3:3 + S].copy()
    A[:, 2] += P[:, 0]
    A[:, 1] += P[:, 1]
    A[:, 0] += P[:, 2]
    A[:, S - 1] += P[:, Sp - 3]
    A[:, S - 2] += P[:, Sp - 2]
    A[:, S - 3] += P[:, Sp - 1]
    return A


def _build_operator_array():
    """Stage-X operator: [3 bands, 2 ktiles, 128 in-rows, 512 out-cols] bf16."""
    A = _build_operator_full()
    return np.ascontiguousarray(
        A.reshape(3, 2, 128, 512).astype(ml_dtypes.bfloat16))


def _build_operator_windows():
    """Stage-Y operator: [3 bands, 4 blocks, 68 in-rows, 128 out-cols] bf16."""
    A = _build_operator_full()
    out = np.zeros((3, 4, KW, 128), np.float64)
    for blk in range(4):
        out[:, blk] = A[:, W0[blk]:W0[blk] + KW, blk * 128:(blk + 1) * 128]
    return np.ascontiguousarray(out.astype(ml_dtypes.bfloat16))


def _build_program(repeat=1):
    nc = bacc.Bacc("TRN2", target_bir_lowering=False)
    # x: [b, p, win, row(+pad)] bf16 — host-materialized overlapping stage-Y
    # windows (row = (by, g2, bx, w) channel-permuted).  Row-interleaved
    # window layout keeps the DMA descriptor stream sequential in DRAM;
    # the 128 B pad stops descriptor coalescing so one load = 280 x 18 KB
    # descriptors -> all 16 SDMA engines at full rate.
    ROW = 3 * 12 * W
    PADW = 64
    x = nc.declare_dram_parameter("x", [BPC, 2, KW, 2, ROW + PADW], BF16,
                                  isOutput=False)
    a_w = nc.declare_dram_parameter("a_w", [3, 4, KW, 128], BF16,
                                    isOutput=False)
    a_op = nc.declare_dram_parameter("a_op", [3, 2, 128, W2], BF16,
                                     isOutput=False)
    ident = nc.declare_dram_parameter("ident", [128, 128], BF16,
                                      isOutput=False)
    out = nc.declare_dram_parameter("out", [BPC, H2, W2, 4], F32,
                                    isOutput=True)

    with tile.TileContext(nc) as tc, ExitStack() as ctx:
        const = ctx.enter_context(tc.tile_pool(name="const", bufs=1))
        xpoolA = ctx.enter_context(tc.tile_pool(name="xpA", bufs=2))
        xpoolB = ctx.enter_context(tc.tile_pool(name="xpB", bufs=1))
        upool = ctx.enter_context(tc.tile_pool(name="up", bufs=2))
        wpool = ctx.enter_context(tc.tile_pool(name="wp", bufs=2))
        vpool = ctx.enter_context(tc.tile_pool(name="vp", bufs=2))
        opool = ctx.enter_context(tc.tile_pool(name="op", bufs=3))
        psY = ctx.enter_context(tc.tile_pool(name="psY", bufs=2, space="PSUM"))
        psT = ctx.enter_context(tc.tile_pool(name="psT", bufs=2, space="PSUM"))
        psX = ctx.enter_context(tc.tile_pool(name="psX", bufs=2, space="PSUM"))
        psO = ctx.enter_context(tc.tile_pool(name="psO", bufs=2, space="PSUM"))

        aw_sb = {}
        for beta in range(3):
            for blk in range(4):
                t = const.tile([KW, 128], BF16, name=f"aw_{beta}_{blk}",
                               tag=f"aw_{beta}_{blk}")
                nc.sync.dma_start(t[:], a_w[beta, blk])
                aw_sb[beta, blk] = t
        a_sb = {}
        for beta in range(3):
            for kt in range(2):
                t = const.tile([128, W2], BF16, name=f"a_{beta}_{kt}",
                               tag=f"a_{beta}_{kt}")
                nc.sync.dma_start(t[:], a_op[beta, kt])
                a_sb[beta, kt] = t
        ident_sb = const.tile([128, 128], BF16, name="ident_sb", tag="ident")
        nc.sync.dma_start(ident_sb[:], ident[:])

        for rep in range(repeat):
          for b in range(BPC):
            rb = rep * BPC + b
            # ---- two DMAs per image (window pairs), 140 descriptors each
            xts = {}
            for pr, pool in ((0, xpoolA), (1, xpoolB)):
                xt = pool.tile([KW, 2 * ROW], BF16, name=f"x_{rb}_{pr}",
                               tag=f"x{pr}")
                src = x[b, pr, :, :, 0:ROW]
                dst = xt.rearrange("h (win r) -> h win r", win=2)
                eng = nc.sync if pr == 0 else nc.scalar
                eng.dma_start(dst, src)
                xts[pr] = xt
            for blk in range(4):
                xv = xts[blk // 2].rearrange(
                    "h (win by g2 bx w) -> h win by g2 bx w",
                    win=2, by=3, g2=4, bx=3)[:, blk % 2]

                # ---- stage Y: u[blk] [h2 128, (g2, bx, w)] bf16
                u = upool.tile([128, 12 * W], BF16, name=f"u_{rb}_{blk}",
                               tag="u")
                uv = u.rearrange("p (g2 bx w) -> p g2 bx w", g2=4, bx=3)
                for bx in range(3):
                    for wc in range(2):
                        ps = psY.tile([128, 512], F32,
                                      name=f"psY_{rb}_{blk}_{bx}_{wc}",
                                      tag="psY")
                        psv = ps.rearrange("p (g w) -> p g w", g=4)
                        for i, by in enumerate(range(3)):
                            rhs = xv[:, by, :, bx, wc * 128:(wc + 1) * 128]
                            nc.tensor.matmul(psv, aw_sb[by, blk][:], rhs,
                                             start=(i == 0), stop=(i == 2))
                        dst = uv[:, :, bx, wc * 128:(wc + 1) * 128]
                        if (bx + wc) % 2 == 0:
                            nc.vector.tensor_copy(out=dst, in_=psv)
                        else:
                            nc.scalar.copy(out=dst, in_=psv)

                # ---- mid transpose: u [h2, (q, w)] -> up[wt] [w, (q, h2)]
                up = {}
                for wt in range(2):
                    up[wt] = wpool.tile([128, 12 * 128], BF16,
                                        name=f"up_{rb}_{blk}_{wt}", tag="upt")
                uvb = u.rearrange("p (q w) -> p q w", q=12)
                for wt in range(2):
                    for q4 in range(3):
                        pt = psT.tile([128, 512], BF16,
                                      name=f"psT_{rb}_{blk}_{wt}_{q4}",
                                      tag="psT")
                        for i in range(4):
                            q = q4 * 4 + i
                            nc.tensor.transpose(
                                pt[:, i * 128:(i + 1) * 128],
                                uvb[:, q, wt * 128:(wt + 1) * 128],
                                ident_sb[:])
                        dst = up[wt].rearrange("p (q h) -> p q h", q=12)[
                            :, q4 * 4:(q4 + 1) * 4, :]
                        src2 = pt.rearrange("p (q h) -> p q h", q=4)
                        if (wt + q4) % 2 == 0:
                            nc.vector.tensor_copy(out=dst, in_=src2)
                        else:
                            nc.scalar.copy(out=dst, in_=src2)

                # ---- stage X for this h2 block -> v[w2b] [w2, (c, h2slice)]
                v = {}
                for w2b in range(4):
                    ps = psX.tile([128, 512], F32,
                                  name=f"psX_{rb}_{blk}_{w2b}", tag="psX")
                    psv = ps.rearrange("p (g h) -> p g h", g=4)
                    mms = [(bx, kt) for bx in range(3) for kt in KTS[w2b]]
                    for i, (bx, kt) in enumerate(mms):
                        lhsT = a_sb[bx, kt][:, w2b * 128:(w2b + 1) * 128]
                        # up free order q = 3*g2 + bx -> fix bx, stride g2
                        rhs = up[kt].rearrange(
                            "p (g2 e h) -> p e g2 h", g2=4, e=3)[:, bx, :, :]
                        nc.tensor.matmul(psv, lhsT, rhs,
                                         start=(i == 0),
                                         stop=(i == len(mms) - 1))
                    vt = vpool.tile([128, 4 * 128], BF16,
                                    name=f"v_{rb}_{blk}_{w2b}",
                                    tag=f"v_{w2b}")
                    dst = vt.rearrange("p (c h) -> p c h", c=4)
                    if w2b % 2 == 0:
                        nc.scalar.copy(out=dst, in_=psv)
                    else:
                        nc.vector.tensor_copy(out=dst, in_=psv)
                    v[w2b] = vt

                # ---- output transpose: v [w2, (c, h2)] -> osb [h2, (w2, c)]
                osb = opool.tile([128, W2 * 4], F32, name=f"osb_{rb}_{blk}",
                                 tag="osb")
                osbv = osb.rearrange("p (w c) -> p c w", c=4)
                for w2b in range(4):
                    pt = psO.tile([128, 512], BF16,
                                  name=f"psO_{rb}_{blk}_{w2b}", tag="psO")
                    vv = v[w2b].rearrange("p (c h) -> p c h", c=4)
                    for c in range(4):
                        nc.tensor.transpose(
                            pt[:, c * 128:(c + 1) * 128],
                            vv[:, c, :],
                            ident_sb[:])
                    dst = osbv[:, :, w2b * 128:(w2b + 1) * 128]
                    src2 = pt.rearrange("p (c w) -> p c w", c=4)
                    if w2b % 2 == 0:
                        nc.vector.tensor_copy(out=dst, in_=src2)
                    else:
                        nc.scalar.copy(out=dst, in_=src2)
                dstd = out[b, blk * 128:(blk + 1) * 128, :, :].rearrange(
                    "h w c -> h (w c)")
                nc.sync.dma_start(dstd, osb[:])
    nc.compile()
    return nc


_PROGRAMS = {}


def _get_program(repeat=1):
    if repeat not in _PROGRAMS:
        _PROGRAMS[repeat] = _build_program(repeat)
    return _PROGRAMS[repeat]


def _host_inputs(inputs):
    a4 = _build_operator_array()
    aw = _build_operator_windows()
    identity = np.ascontiguousarray(np.eye(128, dtype=ml_dtypes.bfloat16))
    # [B,H,W,C] c = 9*g2 + 3*by + bx -> rows [B, H, (by, g2, bx, W)] bf16
    xp = inputs.reshape(B, H, W, 4, 3, 3).transpose(0, 1, 4, 3, 5, 2)
    xp = xp.astype(ml_dtypes.bfloat16).reshape(B, H, 3 * 12 * W)
    ROW = 3 * 12 * W
    PADW = 64
    # materialize the 4 overlapping stage-Y windows, pair-grouped and
    # row-interleaved within each pair, plus pad to stop desc coalescing
    idx = np.arange(KW)[:, None] + np.asarray(W0)[None, :]  # [KW, 4]
    xw = np.zeros((B, 2, KW, 2, ROW + PADW), dtype=ml_dtypes.bfloat16)
    gathered = xp[:, idx, :]  # [B, KW, 4, ROW]
    xw[:, 0, :, :, :ROW] = gathered[:, :, 0:2]
    xw[:, 1, :, :, :ROW] = gathered[:, :, 2:4]
    shards = xw.reshape(NCORES, BPC, 2, KW, 2, ROW + PADW)
    return [{"x": np.ascontiguousarray(shards[c]), "a_op": a4, "a_w": aw,
             "ident": identity} for c in range(NCORES)]


def _run(inputs, trace=False, tmpdir=None, repeat=1):
    """Returns (full output [16,512,512,4], BassKernelResults)."""
    inputs = np.ascontiguousarray(np.asarray(inputs, dtype=np.float32))
    assert inputs.shape == (B, H, W, C), inputs.shape
    nc = _get_program(repeat)
    in_maps = _host_inputs(inputs)
    res = run_bass_kernel_spmd(nc, in_maps, core_ids=list(range(NCORES)),
                               trace=trace, tmpdir=tmpdir)
    outs = [np.asarray(res.results[c]["out"]) for c in range(NCORES)]
    full = np.concatenate(outs, axis=0).astype(np.float32)
    return full, res


def kernel(inputs):
    full, _ = _run(inputs)
    return full
